# revision 19
# baseline (speedup 1.0000x reference)
"""Trainium2 Bass kernel for nn_CubeNet_87093346828907 (loss_fn).

Reference computation, for X:[384,128], G:[128,128], y:[384]:
    M = G^T G;  csmd[i,j] = Mahalanobis-sq cross distances with M (symmetric)
    csmd2 = same with M@M;  denom[j,k] = 2*sqrt(max(csmd2,EPS))
    margins[i,j,k] = relu(csmd[i,k]-csmd[i,j]) / denom[j,k]
    (zeroed where y[i]!=y[j] or i==j)
    inner[i,k] = max_j margins;  out[i] = min over k with y[k]!=y[i]

Sharding: the i-axis is split across 8 NeuronCores (48 rows each); one SPMD
module, per-core behavior driven purely by input data (a 0/1 row-selection
matrix SELT plus mask tensors built on host from y). Each core computes the
full [384,384] csmd/csmd2 on-chip (small PE matmuls), then for each of its
48 i-rows and each 128-wide k-chunk:
  PE:  broadcast the masked csmd row i across 128 partitions (ones outer prod)
  ACT: relu(csmd[i,k_p] - row[j])      (scale=-1, per-partition bias)
  DVE: tensor_tensor_reduce(mult, max, init=0) against 1/denom -> inner[i,k_p]
Finally PE-transposes the [128k,48i] accumulators, adds BIG where y[k]==y[i],
min-reduces over k and DMAs 48 outputs per core.
"""

import sys

import numpy as np

sys.path.insert(0, "/opt/trn_rl_repo")

import concourse.bacc as bacc
import concourse.mybir as mybir
from concourse import masks
from concourse import tile

N = 384
D = 128
NCORES = 8
NSH = N // NCORES  # 48 i-rows per core
P = 128
NKC = N // P  # 3 chunks of 128 along k
BIG = 1.0e30
EPS = 1.0e-6

F32 = mybir.dt.float32
AF = mybir.ActivationFunctionType
ALU = mybir.AluOpType
AX = mybir.AxisListType

# ordered debug truncation points
_STAGES = ["inputs", "csmd", "invd", "cmask", "ccol", "bc2", "act2", "inner2", "full"]


def build_nc(stage="full"):
    """Build the SPMD Bass module (identical for all cores).

    stage: debug truncation point from _STAGES; early stages DMA an
    intermediate to the DBG output and skip everything after.
    """
    lvl = _STAGES.index(stage)
    nc = bacc.Bacc("TRN2", target_bir_lowering=False, debug=False)

    xt_d = nc.dram_tensor("XT", [D, N], F32, kind="ExternalInput")
    g_d = nc.dram_tensor("G", [D, D], F32, kind="ExternalInput")
    selt_d = nc.dram_tensor("SELT", [N, NSH], F32, kind="ExternalInput")
    madd_d = nc.dram_tensor("MADD", [NSH, N], F32, kind="ExternalInput")
    mk_d = nc.dram_tensor("MK", [NSH, N], F32, kind="ExternalInput")
    out_d = nc.dram_tensor("OUT", [NSH, 1], F32, kind="ExternalOutput")
    stage_d = nc.dram_tensor("cmask_stage", [1, NSH * N], F32, kind="Internal")
    dbg_d = (
        nc.dram_tensor("DBG", [P, N], F32, kind="ExternalOutput")
        if stage != "full"
        else None
    )

    def _cut(ap):
        nc.sync.dma_start(dbg_d[: ap.shape[0], : ap.shape[1]], ap)
        nc.sync.dma_start(out_d[:], ap[0:NSH, 0:1])

    with tile.TileContext(nc) as tc:
        with (
            tc.tile_pool(name="singles", bufs=1) as sg,
            tc.tile_pool(name="work", bufs=2) as wk,
            tc.tile_pool(name="relu", bufs=3) as rp,
            tc.tile_pool(name="scr", bufs=2) as sp,
            tc.tile_pool(name="psum_mm", bufs=2, space="PSUM") as ps,
            tc.tile_pool(name="psum_bc", bufs=3, space="PSUM") as psbc,
            tc.tile_pool(name="psum_fin", bufs=1, space="PSUM") as psf,
        ):
            # ---- load inputs ----
            XT = sg.tile([D, N], F32)
            Gt = sg.tile([D, D], F32)
            MADD = sg.tile([NSH, N], F32)
            MK = sg.tile([NSH, N], F32)
            nc.sync.dma_start(XT[:], xt_d[:])
            nc.sync.dma_start(Gt[:], g_d[:])
            nc.sync.dma_start(MADD[:], madd_d[:])
            nc.sync.dma_start(MK[:], mk_d[:])
            selt_ch = selt_d.rearrange("(c p) m -> c p m", p=P)
            SELTc = []
            for ic in range(NKC):
                st = sg.tile([P, NSH], F32, name=f"selt{ic}", tag=f"selt{ic}")
                nc.sync.dma_start(st[:], selt_ch[ic])
                SELTc.append(st)

            ones = sg.tile([P, P], F32)
            nc.vector.memset(ones[:], 1.0)
            ident = sg.tile([P, P], F32)
            masks.make_identity(nc, ident[:])

            if lvl >= 1:
                # ---- M = G^T G ; M2 = M M (both symmetric) ----
                m_ps = ps.tile([P, D], F32, tag="mm")
                nc.tensor.matmul(m_ps[:], Gt[:], Gt[:])
                Mt = sg.tile([P, D], F32)
                nc.scalar.copy(Mt[:], m_ps[:])

                m2_ps = ps.tile([P, D], F32, tag="mm")
                nc.tensor.matmul(m2_ps[:], Mt[:], Mt[:])
                M2t = sg.tile([P, D], F32)
                nc.scalar.copy(M2t[:], m2_ps[:])

                # ---- XMT = M @ XT ([D,N]), XM2T = M2 @ XT ----
                xmt_ps = ps.tile([D, N], F32, tag="mm")
                nc.tensor.matmul(xmt_ps[:], Mt[:], XT[:])
                XMT = sg.tile([D, N], F32)
                nc.scalar.copy(XMT[:], xmt_ps[:])

                xm2t_ps = ps.tile([D, N], F32, tag="mm")
                nc.tensor.matmul(xm2t_ps[:], M2t[:], XT[:])
                XM2T = sg.tile([D, N], F32)
                nc.scalar.copy(XM2T[:], xm2t_ps[:])

                # ---- prod[i] = sum_d XMT[d,i]*XT[d,i] ----
                def prod_vecs(XMTt, tag):
                    tmp = wk.tile([D, N], F32, tag="tmp", name=f"tmp{tag}")
                    nc.vector.tensor_tensor(tmp[:], XMTt[:], XT[:], ALU.mult)
                    prow_ps = ps.tile([1, N], F32, tag="mm", name=f"prowps{tag}")
                    nc.tensor.matmul(prow_ps[:], ones[:, 0:1], tmp[:])
                    prow = sg.tile([1, N], F32, tag=f"prowsb{tag}", name=f"prow{tag}")
                    nc.scalar.copy(prow[:], prow_ps[:])
                    pcol = sg.tile([P, NKC], F32, tag=f"pcol{tag}", name=f"pcol{tag}")
                    for ic in range(NKC):
                        pc_ps = ps.tile([P, 1], F32, tag="mm", name=f"pcps{tag}{ic}")
                        nc.tensor.matmul(
                            pc_ps[:], tmp[:, ic * P : (ic + 1) * P], ones[:, 0:1]
                        )
                        nc.scalar.copy(pcol[:, ic : ic + 1], pc_ps[:])
                    pb_ps = ps.tile([P, N], F32, tag="mm", name=f"pbps{tag}")
                    nc.tensor.matmul(pb_ps[:], ones[0:1, :], prow[:])
                    pb = sg.tile([P, N], F32, tag=f"pb{tag}", name=f"pb{tag}")
                    nc.scalar.copy(pb[:], pb_ps[:])
                    return pcol, pb

                pcol, pb = prod_vecs(XMT, "1")
                pcol2, pb2 = prod_vecs(XM2T, "2")

                # ---- csmd chunks + invd = 1/sqrt(4*max(csmd2,EPS)) ----
                csmd_c = []
                invd_c = []
                for ic in range(NKC):
                    cr_ps = ps.tile([P, N], F32, tag="mm", name=f"crps{ic}")
                    nc.tensor.matmul(cr_ps[:], XMT[:, ic * P : (ic + 1) * P], XT[:])
                    t1 = wk.tile([P, N], F32, tag="t1", name=f"t1_{ic}")
                    nc.vector.tensor_scalar(
                        t1[:], cr_ps[:], -2.0, pcol[:, ic : ic + 1], ALU.mult, ALU.add
                    )
                    cs = sg.tile([P, N], F32, tag=f"csmd{ic}", name=f"csmd{ic}")
                    nc.vector.tensor_tensor(cs[:], t1[:], pb[:], ALU.add)
                    csmd_c.append(cs)

                    cr2_ps = ps.tile([P, N], F32, tag="mm", name=f"cr2ps{ic}")
                    nc.tensor.matmul(cr2_ps[:], XM2T[:, ic * P : (ic + 1) * P], XT[:])
                    t2 = wk.tile([P, N], F32, tag="t2", name=f"t2_{ic}")
                    nc.vector.tensor_scalar(
                        t2[:], cr2_ps[:], -2.0, pcol2[:, ic : ic + 1], ALU.mult, ALU.add
                    )
                    c2 = wk.tile([P, N], F32, tag="c2", name=f"c2_{ic}")
                    nc.vector.tensor_tensor(c2[:], t2[:], pb2[:], ALU.add)
                    nc.vector.tensor_scalar_max(c2[:], c2[:], EPS)
                    sq = wk.tile([P, N], F32, tag="sq", name=f"sq{ic}")
                    nc.scalar.activation(sq[:], c2[:], AF.Sqrt, scale=4.0)
                    iv = sg.tile([P, N], F32, tag=f"invd{ic}", name=f"invd{ic}")
                    nc.vector.reciprocal(iv[:], sq[:])
                    invd_c.append(iv)

                if stage == "csmd":
                    _cut(csmd_c[0][:])
                if stage == "invd":
                    _cut(invd_c[0][:])

            if lvl >= 3:
                # ---- shard row selection: crows[il,:] = csmd[i0+il,:] ----
                crows_ps = psf.tile([NSH, N], F32, tag="crows")
                for ic in range(NKC):
                    nc.tensor.matmul(
                        crows_ps[:],
                        SELTc[ic][:],
                        csmd_c[ic][:],
                        start=(ic == 0),
                        stop=(ic == NKC - 1),
                    )
                crows = sg.tile([NSH, N], F32)
                nc.scalar.copy(crows[:], crows_ps[:])
                # masked rows for the j-side; raw columns for the k-side bias
                cmask = sg.tile([NSH, N], F32)
                nc.vector.tensor_tensor(cmask[:], crows[:], MADD[:], ALU.add)
                # flatten the 48 masked rows onto one partition (via DRAM) so
                # PE can broadcast any row with a base-partition-0 rhs
                nc.sync.dma_start(
                    stage_d[0:1, :].rearrange("one (p f) -> p f", p=NSH), cmask[:]
                )
                cmask_flat = sg.tile([1, NSH * N], F32)
                nc.sync.dma_start(cmask_flat[:], stage_d[:])
                if stage == "cmask":
                    _cut(cmask[:])

            if lvl >= 4:
                CCOL = []
                for kc in range(NKC):
                    cc_ps = psf.tile([P, NSH], F32, tag="ccps", name=f"ccps{kc}")
                    nc.tensor.transpose(
                        cc_ps[:], crows[:, kc * P : (kc + 1) * P], ident[0:NSH, 0:NSH]
                    )
                    cc = sg.tile([P, NSH], F32, tag=f"ccol{kc}", name=f"ccol{kc}")
                    nc.scalar.copy(cc[:], cc_ps[:])
                    CCOL.append(cc)
                if stage == "ccol":
                    _cut(CCOL[0][:])

            if lvl >= 5:
                # ---- inner loop over the shard rows ----
                innerT = (
                    [
                        sg.tile([P, NSH], F32, name=f"innerT{kc}", tag=f"innerT{kc}")
                        for kc in range(NKC)
                    ]
                    if stage not in ("bc2", "act2")
                    else None
                )
                niter = 1 if stage in ("bc2", "act2") else (2 if stage == "inner2" else NSH)
                for il in range(niter):
                    bc_ps = psbc.tile([P, N], F32, name="bc_ps")
                    nc.tensor.matmul(
                        bc_ps[:], ones[0:1, :], cmask_flat[0:1, il * N : (il + 1) * N]
                    )
                    if stage == "bc2":
                        bdbg = sg.tile([P, N], F32, name="bdbg")
                        nc.scalar.copy(bdbg[:], bc_ps[:])
                        _cut(bdbg[:])
                        continue
                    nkc_run = 1 if stage == "act2" else NKC
                    for kc in range(nkc_run):
                        r_t = rp.tile([P, N], F32, name="r_t")
                        nc.scalar.activation(
                            r_t[:],
                            bc_ps[:],
                            AF.Relu,
                            bias=CCOL[kc][:, il : il + 1],
                            scale=-1.0,
                        )
                        if stage == "act2":
                            _cut(r_t[:])
                            continue
                        scr = sp.tile([P, N], F32, name="scr")
                        nc.vector.tensor_tensor(scr[:], r_t[:], invd_c[kc][:], ALU.mult)
                        nc.vector.tensor_reduce(
                            innerT[kc][:, il : il + 1], scr[:], AX.X, ALU.max
                        )
                if stage == "inner2":
                    _cut(innerT[0][:, 0:2])

            if stage == "full":
                # ---- finalize: transpose back, k-mask, min over k ----
                inner_sh = sg.tile([NSH, N], F32)
                for kc in range(NKC):
                    tr_ps = psf.tile([NSH, P], F32, tag="trps", name=f"trps{kc}")
                    nc.tensor.transpose(tr_ps[:], innerT[kc][:], ident[:])
                    nc.scalar.copy(inner_sh[:, kc * P : (kc + 1) * P], tr_ps[:])
                nc.vector.tensor_tensor(inner_sh[:], inner_sh[:], MK[:], ALU.add)
                out_t = sg.tile([NSH, 1], F32)
                nc.vector.tensor_reduce(out_t[:], inner_sh[:], AX.X, ALU.min)
                nc.sync.dma_start(out_d[:], out_t[:])
            elif stage == "inputs":
                _cut(XT[:])

    nc.compile()
    return nc


def host_prep(X, G, y):
    X = np.asarray(X, dtype=np.float32)
    G = np.asarray(G, dtype=np.float32)
    y = np.asarray(y)
    XT = np.ascontiguousarray(X.T)
    same = y[None, :] == y[:, None]
    madd = np.where(~same | np.eye(N, dtype=bool), BIG, 0.0).astype(np.float32)
    mk = np.where(same, BIG, 0.0).astype(np.float32)
    return XT, G, madd, mk


def make_in_maps(X, G, y):
    XT, Gf, madd, mk = host_prep(X, G, y)
    in_maps = []
    for d in range(NCORES):
        i0 = d * NSH
        selt = np.zeros((N, NSH), dtype=np.float32)
        selt[np.arange(i0, i0 + NSH), np.arange(NSH)] = 1.0
        in_maps.append(
            {
                "XT": XT,
                "G": Gf,
                "SELT": selt,
                "MADD": madd[i0 : i0 + NSH],
                "MK": mk[i0 : i0 + NSH],
            }
        )
    return in_maps


_NC_CACHE = {}


def kernel(X, G, y):
    from concourse.bass_utils import run_bass_kernel_spmd

    if "nc" not in _NC_CACHE:
        _NC_CACHE["nc"] = build_nc()
    in_maps = make_in_maps(X, G, y)
    res = run_bass_kernel_spmd(_NC_CACHE["nc"], in_maps, list(range(NCORES))).results
    out = np.concatenate([res[d]["OUT"].reshape(-1) for d in range(NCORES)])
    return out.astype(np.float32)
